# revision 47
# baseline (speedup 1.0000x reference)
"""Matrix-Tree edge marginals on 8 Trainium2 NeuronCores.

probs[b,i,j] = d logZ / d scores[b,i,j] with logZ from the Matrix-Tree
theorem.  Closed form: with A = exp(masked scores - m) and Lfull the
(row/col-0-padded) Laplacian, probs = A ⊙ (diag(Y)·1^T − Y) where
Y = (Lfull^T)^{-1}.

Device算法 (per 256x256 matrix, 32 per core):
 1. Deflation: the Jacobi-preconditioned Laplacian has ONE slow outlier
    eigenvalue (Perron/root-escape mode) and a tight bulk (|1-λ| ≤ 0.09).
    The host adds γ·mact·mactᵀ (γ = mean_degree/n_active, bf16-exact)
    while building the Laplacian; the true inverse is recovered via a
    rank-1 Sherman-Morrison correction applied on the host.
 2. Host packs the deflated Laplacian Lt and A (it computes exp anyway),
    so device setup is just bf16 splits.
 3. Scaled-space Newton, round 1 in closed form: with G = rt∘Lh (bf16),
    B̄ = I-G, W1 = 2I-G, V1 = W1ᵀ (DMA-XBAR transpose):
    Q = B̄ᵀ@V1 = B+B², Yf1 = rt∘(I+Q) — one 256³ matmul.
 4. Round 2 polishes with the true split-bf16 residual (3-matmul
    Lh/Ll × Yh/Yl product) — needed for Sherman-Morrison denominator
    accuracy (the δ it feeds is a ~5e-3 cancellation).
 5. Device ships Pbase = A⊙(diag(Yf)1ᵀ − Yf) plus the row-sum vector
    u = Ỹ·mact (plain row sums — block-diagonal structure makes masking
    free) and column-sum vector z = mactᵀỸ (split-bf16 thin matmuls).
    Host finishes: δ = 1-γ·z·mact, κ = γ/δ, zk = κz (zk[0]=0: the root
    column of Ỹ is e0), P = Pbase + (A∘u)∘zk_i − (A∘u)∘zk_j.
"""

import numpy as np

import concourse.bass as bass
import concourse.bacc as bacc
import concourse.mybir as mybir
from concourse.bass import ds, ts
from concourse.masks import make_identity
from concourse.tile import TileContext
from concourse.bass_utils import run_bass_kernel_spmd

B, S, P = 256, 256, 128
NCORES = 8
BPC = B // NCORES   # matrices per core
RB = S // P         # row blocks per matrix
GRP = 6             # matrices interleaved per group
CGAMMA = 1.0        # deflation strength
NEG = np.float32(-1e9)

f32 = mybir.dt.float32
bf16 = mybir.dt.bfloat16
MULT = mybir.AluOpType.mult
ADD = mybir.AluOpType.add
SUB = mybir.AluOpType.subtract
AX = mybir.AxisListType.X
COPY = mybir.ActivationFunctionType.Copy
IDENT = mybir.ActivationFunctionType.Identity

OFF_LT = 0                  # RB*S: deflated Laplacian rows
OFF_A = RB * S              # RB*S: A = exp(s - m) rows
OFF_RT = 2 * RB * S         # 2: rt = 1/diag(Lt), column layout
PACK = OFF_RT + 2


def _mm256(nc, out_ps, lhsT, rhs):
    for I in range(RB):
        for K in range(RB):
            nc.tensor.matmul(
                out_ps[:, I, :],
                lhsT[:, K, ts(I, P)],
                rhs[:, K, :],
                start=(K == 0),
                stop=(K == RB - 1),
            )


def _mm256_acc(nc, out_ps, pairs):
    n = len(pairs) * RB
    for I in range(RB):
        cnt = 0
        for lhsT, rhs in pairs:
            for K in range(RB):
                nc.tensor.matmul(
                    out_ps[:, I, :],
                    lhsT[:, K, ts(I, P)],
                    rhs[:, K, :],
                    start=(cnt == 0),
                    stop=(cnt == n - 1),
                )
                cnt += 1


def build_program():
    nc = bacc.Bacc()
    inp = nc.dram_tensor("inp", [BPC, P, PACK], f32, kind="ExternalInput")
    out = nc.dram_tensor("pbase", [BPC, S, S], f32, kind="ExternalOutput")
    uv = nc.dram_tensor("uv", [BPC, P, RB], f32, kind="ExternalOutput")
    zv = nc.dram_tensor("zv", [BPC, 1, S], f32, kind="ExternalOutput")

    with TileContext(nc) as tc:
        with (
            tc.tile_pool(name="consts", bufs=1) as consts,
            tc.tile_pool(name="mat", bufs=3) as mat,
            tc.tile_pool(name="small", bufs=12) as small,
            tc.tile_pool(name="psT", bufs=2, space="PSUM") as ppT,
            tc.tile_pool(name="psD", bufs=2, space="PSUM") as ppD,
            tc.tile_pool(name="psbt", bufs=2, space="PSUM") as pbt,
            tc.tile_pool(name="psrow", bufs=2, space="PSUM") as prow,
        ):
            ident = consts.tile([P, P], f32)
            make_identity(nc, ident)
            identbig = consts.tile([P, RB, S], f32)
            nc.vector.memset(identbig, 0.0)
            for rb in range(RB):
                nc.vector.tensor_copy(identbig[:, rb, ts(rb, P)], ident)
            identbig_bf = consts.tile([P, RB, S], bf16)
            nc.scalar.activation(identbig_bf, identbig, COPY)
            i2bf = consts.tile([P, RB, S], bf16)
            nc.vector.tensor_scalar_mul(i2bf, identbig, 2.0)
            idbf = consts.tile([P, P], bf16)
            nc.scalar.activation(idbf, ident, COPY)
            onescol_bf = consts.tile([P, 1], bf16)
            nc.vector.memset(onescol_bf, 1.0)

            def setup(b):
                st = {}
                packed = mat.tile([P, PACK], f32, tag="packed", bufs=13)
                nc.sync.dma_start(packed, inp[b])
                st["packed"] = packed
                Ltp = packed[:, OFF_LT : OFF_LT + RB * S].rearrange(
                    "p (rb j) -> p rb j", rb=RB
                )
                st["Aa"] = packed[:, OFF_A : OFF_A + RB * S].rearrange(
                    "p (rb j) -> p rb j", rb=RB
                )
                rt = packed[:, OFF_RT : OFF_RT + 2]
                st["rt"] = rt

                Lh = mat.tile([P, RB, S], bf16, tag="Lh", bufs=13)
                nc.scalar.activation(Lh, Ltp, COPY)
                Ll = mat.tile([P, RB, S], bf16, tag="Ll", bufs=13)
                nc.gpsimd.tensor_sub(Ll, Ltp, Lh)
                st["Lh"], st["Ll"] = Lh, Ll

                G = mat.tile([P, RB, S], bf16, tag="G", bufs=7)
                for rb in range(RB):
                    nc.vector.tensor_scalar_mul(
                        G[:, rb, :], Lh[:, rb, :], rt[:, ds(rb, 1)]
                    )
                Bbar = mat.tile([P, RB, S], bf16, tag="Bbar", bufs=7)
                nc.vector.tensor_sub(Bbar, identbig_bf, G)
                W1 = mat.tile([P, RB, S], bf16, tag="W1", bufs=13)
                nc.gpsimd.tensor_sub(W1, i2bf, G)
                st["Bbar"], st["W1"] = Bbar, W1
                # V1 = W1^T via PE transpose (PSUM) + ACT copy to SBUF
                V1ps = pbt.tile([P, RB, S], bf16, tag="BT")
                for I in range(RB):
                    for K in range(RB):
                        nc.tensor.transpose(
                            V1ps[:, I, ts(K, P)], W1[:, K, ts(I, P)], idbf
                        )
                V1sb = mat.tile([P, RB, S], bf16, tag="V1", bufs=7)
                nc.scalar.activation(V1sb, V1ps, COPY)
                st["V1"] = V1sb
                return st

            def round1(st):
                rt = st["rt"]
                Qps = ppD.tile([P, RB, S], f32, tag="dY")
                _mm256(nc, Qps, st["Bbar"], st["V1"])
                S1 = mat.tile([P, RB, S], f32, tag="Yf", bufs=13)
                nc.vector.tensor_add(S1, identbig, Qps)
                for rb in range(RB):
                    nc.scalar.mul(S1[:, rb, :], S1[:, rb, :], rt[:, ds(rb, 1)])
                st["Yf"] = S1

            def round2a(st):
                Yh2 = mat.tile([P, RB, S], bf16, tag="Yh", bufs=8)
                nc.scalar.activation(Yh2, st["Yf"], COPY)
                Yl2 = mat.tile([P, RB, S], bf16, tag="Yl", bufs=8)
                nc.gpsimd.tensor_sub(Yl2, st["Yf"], Yh2)
                Tps = ppT.tile([P, RB, S], f32, tag="T")
                _mm256_acc(
                    nc, Tps,
                    [(st["Lh"], Yh2), (st["Lh"], Yl2), (st["Ll"], Yh2)],
                )
                st["Tps"] = Tps

            def round2b(st):
                rt = st["rt"]
                R2 = mat.tile([P, RB, S], bf16, tag="R", bufs=4)
                nc.vector.tensor_sub(R2, identbig, st["Tps"])
                dY2ps = ppD.tile([P, RB, S], f32, tag="dY")
                _mm256(nc, dY2ps, st["W1"], R2)
                tupd = mat.tile([P, RB, S], f32, tag="tupd", bufs=4)
                for rb in range(RB):
                    nc.scalar.mul(tupd[:, rb, :], dY2ps[:, rb, :], rt[:, ds(rb, 1)])
                nc.gpsimd.tensor_add(st["Yf"], st["Yf"], tupd)

            def sm_out(b, st):
                Yf, Aa = st["Yf"], st["Aa"]
                # z = onesᵀYf via split-bf16 thin matmuls (PSUM accumulates)
                Yh3 = mat.tile([P, RB, S], bf16, tag="Yh", bufs=8)
                nc.scalar.activation(Yh3, Yf, COPY)
                Yl3 = mat.tile([P, RB, S], bf16, tag="Yl", bufs=8)
                nc.vector.tensor_sub(Yl3, Yf, Yh3)
                zw = prow.tile([1, 2 * S], f32, tag="srow")
                nc.tensor.matmul(
                    zw, onescol_bf, Yh3.rearrange("p rb j -> p (rb j)"),
                    start=True, stop=False,
                )
                nc.tensor.matmul(
                    zw, onescol_bf, Yl3.rearrange("p rb j -> p (rb j)"),
                    start=False, stop=True,
                )
                zwsb = small.tile([1, 2 * S], f32, tag="zwsb", bufs=3)
                nc.scalar.activation(zwsb, zw, COPY)
                # u = row sums; z row copied to SBUF for DMA
                uz = small.tile([P, RB], f32, tag="uz", bufs=5)
                nc.vector.tensor_reduce(uz, Yf, AX, ADD)
                zsb = small.tile([1, S], f32, tag="zsb", bufs=5)
                nc.vector.tensor_add(zsb, zwsb[0:1, 0:S], zwsb[0:1, S : 2 * S])
                # dg = diag(Yf); Pbase = A ⊙ (dg_i - Yf)
                dg = small.tile([P, RB], f32, tag="dg")
                for rb in range(RB):
                    scr = small.tile([P, P], f32, tag="scr", bufs=4)
                    nc.gpsimd.tensor_mul(scr, ident, Yf[:, rb, ts(rb, P)])
                    nc.vector.tensor_reduce(dg[:, ds(rb, 1)], scr, AX, ADD)
                t3 = mat.tile([P, RB, S], f32, tag="t3", bufs=4)
                for rb in range(RB):
                    nc.scalar.activation(
                        t3[:, rb, :], Yf[:, rb, :], IDENT,
                        bias=dg[:, ds(rb, 1)], scale=-1.0,
                    )
                Pr = mat.tile([P, RB, S], f32, tag="Pr", bufs=5)
                if b % 2 == 0:
                    nc.vector.tensor_mul(Pr, t3, Aa)
                else:
                    nc.gpsimd.tensor_mul(Pr, t3, Aa)
                nc.sync.dma_start(
                    out[b].rearrange("(rb p) j -> p rb j", p=P), Pr
                )
                nc.sync.dma_start(uv[b], uz)
                nc.sync.dma_start(zv[b], zsb[0:1, :])

            groups = [
                list(range(g0, min(g0 + GRP, BPC)))
                for g0 in range(0, BPC, GRP)
            ]
            sts = {}
            for b in groups[0]:
                sts[b] = setup(b)
            for gi, grp in enumerate(groups):
                nxt = groups[gi + 1] if gi + 1 < len(groups) else []
                for b in grp:
                    round1(sts[b])
                for b in grp:
                    round2a(sts[b])
                for b in grp:
                    round2b(sts[b])
                for b in nxt:
                    sts[b] = setup(b)
                for b in grp:
                    sm_out(b, sts[b])
                    del sts[b]
    nc.finalize()
    return nc


_prog = None


def _get_program():
    global _prog
    if _prog is None:
        _prog = build_program()
    return _prog


def _bf16_exact(x):
    u = np.asarray(x, dtype=np.float32).view(np.uint32)
    u = (u + 0x8000) & 0xFFFF0000
    return u.view(np.float32)


def _host_prep(scores, mask):
    scores = np.asarray(scores, dtype=np.float32)
    mask = np.asarray(mask).astype(bool)
    mr = mask.copy()
    mr[:, 0] = True
    pair = mr[:, :, None] & mr[:, None, :]
    spre = np.where(pair, scores, NEG)
    spre[:, 0, :] = NEG
    m = spre.max(axis=(1, 2))                      # [B]
    E = np.exp(np.clip(spre - m[:, None, None], -80.0, 0.0), dtype=np.float32)
    E[:, 0, :] = 0.0
    d = E.sum(axis=2)                              # [B, S]
    mactf = mask.astype(np.float32)
    n_act = mactf.sum(axis=1)
    dbar = (d * mactf).sum(axis=1) / n_act
    gamma = _bf16_exact(CGAMMA * dbar / n_act)     # [B], bf16-exact

    Lt = -E.copy()
    idx = np.arange(S)
    Lt[:, idx, idx] += d
    Lt += gamma[:, None, None] * (mactf[:, :, None] * mactf[:, None, :])
    Lt = np.where(mr[:, :, None], Lt, np.eye(S, dtype=np.float32)[None])
    Lt[:, :, 0] = 0.0
    Lt[:, 0, :] = 0.0
    Lt[:, 0, 0] = 1.0
    Lt = Lt.astype(np.float32)
    diagL = np.einsum('bii->bi', Lt)
    rt = (np.float32(1.0) / diagL).astype(np.float32)

    def colmaj(v):
        return v.reshape(B, RB, P).transpose(0, 2, 1)

    def rowpack(M):
        return M.reshape(B, RB, P, S).transpose(0, 2, 1, 3).reshape(B, P, RB * S)

    packed = np.zeros((B, P, PACK), dtype=np.float32)
    packed[:, :, OFF_LT : OFF_LT + RB * S] = rowpack(Lt)
    packed[:, :, OFF_A : OFF_A + RB * S] = rowpack(E)
    packed[:, :, OFF_RT : OFF_RT + 2] = colmaj(rt)
    return packed, E, mactf, gamma


def kernel(scores, mask):
    packed, E, mactf, gamma = _host_prep(scores, mask)
    nc = _get_program()
    in_maps = [
        {"inp": packed[i * BPC:(i + 1) * BPC]}
        for i in range(NCORES)
    ]
    res = run_bass_kernel_spmd(nc, in_maps, list(range(NCORES)))
    pbase = np.concatenate(
        [res.results[i]["pbase"] for i in range(NCORES)], axis=0
    ).astype(np.float32)
    u = np.concatenate(
        [res.results[i]["uv"] for i in range(NCORES)], axis=0
    ).astype(np.float32).transpose(0, 2, 1).reshape(B, S)
    z = np.concatenate(
        [res.results[i]["zv"] for i in range(NCORES)], axis=0
    ).astype(np.float32).reshape(B, S)
    # host Sherman-Morrison combine (f32)
    sdot = (z * mactf).sum(axis=1)
    delta = np.float32(1.0) - gamma * sdot
    kappa = (gamma / delta).astype(np.float32)
    zk = kappa[:, None] * z
    zk[:, 0] = 0.0
    Au = E * u[:, :, None]
    probs = pbase + Au * zk[:, :, None] - Au * zk[:, None, :]
    return probs.astype(np.float32)


# revision 50
# speedup vs baseline: 1.0144x; 1.0144x over previous
"""Matrix-Tree edge marginals on 8 Trainium2 NeuronCores.

probs[b,i,j] = d logZ / d scores[b,i,j] with logZ from the Matrix-Tree
theorem.  Closed form: with A = exp(masked scores - m) and Lfull the
(row/col-0-padded) Laplacian, probs = A ⊙ (diag(Y)·1^T − Y) where
Y = (Lfull^T)^{-1}.

Device算法 (per 256x256 matrix, 32 per core):
 1. Deflation: the Jacobi-preconditioned Laplacian has ONE slow outlier
    eigenvalue (Perron/root-escape mode) and a tight bulk (|1-λ| ≤ 0.09).
    The host adds γ·mact·mactᵀ (γ = mean_degree/n_active, bf16-exact)
    while building the Laplacian; the true inverse is recovered via a
    rank-1 Sherman-Morrison correction applied on the host.
 2. Host packs the deflated Laplacian Lt and A (it computes exp anyway),
    so device setup is just bf16 splits.
 3. Scaled-space Newton, round 1 in closed form: with G = rt∘Lh (bf16),
    B̄ = I-G, W1 = 2I-G, V1 = W1ᵀ (DMA-XBAR transpose):
    Q = B̄ᵀ@V1 = B+B², Yf1 = rt∘(I+Q) — one 256³ matmul.
 4. Round 2 polishes with the true split-bf16 residual (3-matmul
    Lh/Ll × Yh/Yl product) — needed for Sherman-Morrison denominator
    accuracy (the δ it feeds is a ~5e-3 cancellation).
 5. Device ships Pbase = A⊙(diag(Yf)1ᵀ − Yf) plus the row-sum vector
    u = Ỹ·mact (plain row sums — block-diagonal structure makes masking
    free) and column-sum vector z = mactᵀỸ (split-bf16 thin matmuls).
    Host finishes: δ = 1-γ·z·mact, κ = γ/δ, zk = κz (zk[0]=0: the root
    column of Ỹ is e0), P = Pbase + (A∘u)∘zk_i − (A∘u)∘zk_j.
"""

import numpy as np

import concourse.bass as bass
import concourse.bacc as bacc
import concourse.mybir as mybir
from concourse.bass import ds, ts
from concourse.masks import make_identity
from concourse.tile import TileContext
from concourse.bass_utils import run_bass_kernel_spmd

B, S, P = 256, 256, 128
NCORES = 8
BPC = B // NCORES   # matrices per core
RB = S // P         # row blocks per matrix
GRP = 6             # matrices interleaved per group
CGAMMA = 1.0        # deflation strength
NEG = np.float32(-1e9)

f32 = mybir.dt.float32
bf16 = mybir.dt.bfloat16
MULT = mybir.AluOpType.mult
ADD = mybir.AluOpType.add
SUB = mybir.AluOpType.subtract
AX = mybir.AxisListType.X
COPY = mybir.ActivationFunctionType.Copy
IDENT = mybir.ActivationFunctionType.Identity

OFF_LT = 0                  # RB*S: deflated Laplacian rows
OFF_A = RB * S              # RB*S: A = exp(s - m) rows
OFF_RT = 2 * RB * S         # 2: rt = 1/diag(Lt), column layout
PACK = OFF_RT + 2


def _mm256(nc, out_ps, lhsT, rhs):
    for I in range(RB):
        for K in range(RB):
            nc.tensor.matmul(
                out_ps[:, I, :],
                lhsT[:, K, ts(I, P)],
                rhs[:, K, :],
                start=(K == 0),
                stop=(K == RB - 1),
            )


def _mm256_acc(nc, out_ps, pairs):
    n = len(pairs) * RB
    for I in range(RB):
        cnt = 0
        for lhsT, rhs in pairs:
            for K in range(RB):
                nc.tensor.matmul(
                    out_ps[:, I, :],
                    lhsT[:, K, ts(I, P)],
                    rhs[:, K, :],
                    start=(cnt == 0),
                    stop=(cnt == n - 1),
                )
                cnt += 1


def build_program():
    nc = bacc.Bacc()
    inp = nc.dram_tensor("inp", [BPC, P, PACK], f32, kind="ExternalInput")
    out = nc.dram_tensor("pbase", [BPC, S, S], f32, kind="ExternalOutput")
    uv = nc.dram_tensor("uv", [BPC, P, RB], f32, kind="ExternalOutput")
    zv = nc.dram_tensor("zv", [BPC, 1, S], f32, kind="ExternalOutput")

    with TileContext(nc) as tc:
        with (
            tc.tile_pool(name="consts", bufs=1) as consts,
            tc.tile_pool(name="mat", bufs=3) as mat,
            tc.tile_pool(name="small", bufs=12) as small,
            tc.tile_pool(name="psT", bufs=2, space="PSUM") as ppT,
            tc.tile_pool(name="psD", bufs=3, space="PSUM") as ppD,
            tc.tile_pool(name="psbt", bufs=1, space="PSUM") as pbt,
            tc.tile_pool(name="psrow", bufs=2, space="PSUM") as prow,
        ):
            ident = consts.tile([P, P], f32)
            make_identity(nc, ident)
            identbig = consts.tile([P, RB, S], f32)
            nc.vector.memset(identbig, 0.0)
            for rb in range(RB):
                nc.vector.tensor_copy(identbig[:, rb, ts(rb, P)], ident)
            identbig_bf = consts.tile([P, RB, S], bf16)
            nc.scalar.activation(identbig_bf, identbig, COPY)
            i2bf = consts.tile([P, RB, S], bf16)
            nc.vector.tensor_scalar_mul(i2bf, identbig, 2.0)
            idbf = consts.tile([P, P], bf16)
            nc.scalar.activation(idbf, ident, COPY)
            onescol_bf = consts.tile([P, 1], bf16)
            nc.vector.memset(onescol_bf, 1.0)

            def setup(b):
                st = {}
                packed = mat.tile([P, PACK], f32, tag="packed", bufs=13)
                nc.sync.dma_start(packed, inp[b])
                st["packed"] = packed
                Ltp = packed[:, OFF_LT : OFF_LT + RB * S].rearrange(
                    "p (rb j) -> p rb j", rb=RB
                )
                st["Aa"] = packed[:, OFF_A : OFF_A + RB * S].rearrange(
                    "p (rb j) -> p rb j", rb=RB
                )
                rt = packed[:, OFF_RT : OFF_RT + 2]
                st["rt"] = rt

                Lh = mat.tile([P, RB, S], bf16, tag="Lh", bufs=13)
                nc.scalar.activation(Lh, Ltp, COPY)
                Ll = mat.tile([P, RB, S], bf16, tag="Ll", bufs=13)
                nc.gpsimd.tensor_sub(Ll, Ltp, Lh)
                st["Lh"], st["Ll"] = Lh, Ll

                G = mat.tile([P, RB, S], bf16, tag="G", bufs=7)
                for rb in range(RB):
                    nc.vector.tensor_scalar_mul(
                        G[:, rb, :], Lh[:, rb, :], rt[:, ds(rb, 1)]
                    )
                Bbar = mat.tile([P, RB, S], bf16, tag="Bbar", bufs=7)
                nc.vector.tensor_sub(Bbar, identbig_bf, G)
                W1 = mat.tile([P, RB, S], bf16, tag="W1", bufs=13)
                nc.gpsimd.tensor_sub(W1, i2bf, G)
                st["Bbar"], st["W1"] = Bbar, W1
                # V1 = W1^T via PE transpose (PSUM) + ACT copy to SBUF
                V1ps = pbt.tile([P, RB, S], bf16, tag="BT")
                for I in range(RB):
                    for K in range(RB):
                        nc.tensor.transpose(
                            V1ps[:, I, ts(K, P)], W1[:, K, ts(I, P)], idbf
                        )
                V1sb = mat.tile([P, RB, S], bf16, tag="V1", bufs=7)
                nc.scalar.activation(V1sb, V1ps, COPY)
                st["V1"] = V1sb
                return st

            def round1(st):
                rt = st["rt"]
                Qps = ppD.tile([P, RB, S], f32, tag="dY")
                _mm256(nc, Qps, st["Bbar"], st["V1"])
                S1 = mat.tile([P, RB, S], f32, tag="Yf", bufs=13)
                nc.vector.tensor_add(S1, identbig, Qps)
                for rb in range(RB):
                    nc.scalar.mul(S1[:, rb, :], S1[:, rb, :], rt[:, ds(rb, 1)])
                st["Yf"] = S1

            def round2a(st):
                Yh2 = mat.tile([P, RB, S], bf16, tag="Yh", bufs=8)
                nc.scalar.activation(Yh2, st["Yf"], COPY)
                Yl2 = mat.tile([P, RB, S], bf16, tag="Yl", bufs=8)
                nc.gpsimd.tensor_sub(Yl2, st["Yf"], Yh2)
                Tps = ppT.tile([P, RB, S], f32, tag="T")
                _mm256_acc(
                    nc, Tps,
                    [(st["Lh"], Yh2), (st["Lh"], Yl2), (st["Ll"], Yh2)],
                )
                st["Tps"] = Tps

            def round2b(st):
                rt = st["rt"]
                R2 = mat.tile([P, RB, S], bf16, tag="R", bufs=4)
                nc.vector.tensor_sub(R2, identbig, st["Tps"])
                dY2ps = ppD.tile([P, RB, S], f32, tag="dY")
                _mm256(nc, dY2ps, st["W1"], R2)
                tupd = mat.tile([P, RB, S], f32, tag="tupd", bufs=4)
                for rb in range(RB):
                    nc.scalar.mul(tupd[:, rb, :], dY2ps[:, rb, :], rt[:, ds(rb, 1)])
                nc.gpsimd.tensor_add(st["Yf"], st["Yf"], tupd)

            def sm_out(b, st):
                Yf, Aa = st["Yf"], st["Aa"]
                # z = onesᵀYf via split-bf16 thin matmuls (PSUM accumulates)
                Yh3 = mat.tile([P, RB, S], bf16, tag="Yh", bufs=8)
                nc.scalar.activation(Yh3, Yf, COPY)
                Yl3 = mat.tile([P, RB, S], bf16, tag="Yl", bufs=8)
                nc.vector.tensor_sub(Yl3, Yf, Yh3)
                zps = prow.tile([1, S], f32, tag="srow")
                cnt = 0
                for piece in (Yh3, Yl3):
                    for rb in range(RB):
                        nc.tensor.matmul(
                            zps, onescol_bf, piece[:, rb, :],
                            start=(cnt == 0), stop=(cnt == 2 * RB - 1),
                        )
                        cnt += 1
                # u = row sums; z row copied to SBUF for DMA
                uz = small.tile([P, RB], f32, tag="uz", bufs=5)
                nc.vector.tensor_reduce(uz, Yf, AX, ADD)
                zsb = small.tile([1, S], f32, tag="zsb", bufs=5)
                nc.scalar.activation(zsb, zps, COPY)
                # dg = diag(Yf); Pbase = A ⊙ (dg_i - Yf)
                dg = small.tile([P, RB], f32, tag="dg")
                for rb in range(RB):
                    scr = small.tile([P, P], f32, tag="scr", bufs=4)
                    nc.gpsimd.tensor_mul(scr, ident, Yf[:, rb, ts(rb, P)])
                    nc.vector.tensor_reduce(dg[:, ds(rb, 1)], scr, AX, ADD)
                t3 = mat.tile([P, RB, S], f32, tag="t3", bufs=4)
                for rb in range(RB):
                    nc.scalar.activation(
                        t3[:, rb, :], Yf[:, rb, :], IDENT,
                        bias=dg[:, ds(rb, 1)], scale=-1.0,
                    )
                Pr = mat.tile([P, RB, S], f32, tag="Pr", bufs=5)
                if b % 2 == 0:
                    nc.vector.tensor_mul(Pr, t3, Aa)
                else:
                    nc.gpsimd.tensor_mul(Pr, t3, Aa)
                nc.sync.dma_start(
                    out[b].rearrange("(rb p) j -> p rb j", p=P), Pr
                )
                nc.sync.dma_start(uv[b], uz)
                nc.sync.dma_start(zv[b], zsb[0:1, :])

            groups = [
                list(range(g0, min(g0 + GRP, BPC)))
                for g0 in range(0, BPC, GRP)
            ]
            sts = {}
            for b in groups[0]:
                sts[b] = setup(b)
            for gi, grp in enumerate(groups):
                nxt = groups[gi + 1] if gi + 1 < len(groups) else []
                for b in grp:
                    round1(sts[b])
                for b in grp:
                    round2a(sts[b])
                for b in grp:
                    round2b(sts[b])
                for b in nxt:
                    sts[b] = setup(b)
                for b in grp:
                    sm_out(b, sts[b])
                    del sts[b]
    nc.finalize()
    return nc


_prog = None


def _get_program():
    global _prog
    if _prog is None:
        _prog = build_program()
    return _prog


def _bf16_exact(x):
    u = np.asarray(x, dtype=np.float32).view(np.uint32)
    u = (u + 0x8000) & 0xFFFF0000
    return u.view(np.float32)


def _host_prep(scores, mask):
    scores = np.asarray(scores, dtype=np.float32)
    mask = np.asarray(mask).astype(bool)
    mr = mask.copy()
    mr[:, 0] = True
    pair = mr[:, :, None] & mr[:, None, :]
    spre = np.where(pair, scores, NEG)
    spre[:, 0, :] = NEG
    m = spre.max(axis=(1, 2))                      # [B]
    E = np.exp(np.clip(spre - m[:, None, None], -80.0, 0.0), dtype=np.float32)
    E[:, 0, :] = 0.0
    d = E.sum(axis=2)                              # [B, S]
    mactf = mask.astype(np.float32)
    n_act = mactf.sum(axis=1)
    dbar = (d * mactf).sum(axis=1) / n_act
    gamma = _bf16_exact(CGAMMA * dbar / n_act)     # [B], bf16-exact

    Lt = -E.copy()
    idx = np.arange(S)
    Lt[:, idx, idx] += d
    Lt += gamma[:, None, None] * (mactf[:, :, None] * mactf[:, None, :])
    Lt = np.where(mr[:, :, None], Lt, np.eye(S, dtype=np.float32)[None])
    Lt[:, :, 0] = 0.0
    Lt[:, 0, :] = 0.0
    Lt[:, 0, 0] = 1.0
    Lt = Lt.astype(np.float32)
    diagL = np.einsum('bii->bi', Lt)
    rt = (np.float32(1.0) / diagL).astype(np.float32)

    def colmaj(v):
        return v.reshape(B, RB, P).transpose(0, 2, 1)

    def rowpack(M):
        return M.reshape(B, RB, P, S).transpose(0, 2, 1, 3).reshape(B, P, RB * S)

    packed = np.zeros((B, P, PACK), dtype=np.float32)
    packed[:, :, OFF_LT : OFF_LT + RB * S] = rowpack(Lt)
    packed[:, :, OFF_A : OFF_A + RB * S] = rowpack(E)
    packed[:, :, OFF_RT : OFF_RT + 2] = colmaj(rt)
    return packed, E, mactf, gamma


def kernel(scores, mask):
    packed, E, mactf, gamma = _host_prep(scores, mask)
    nc = _get_program()
    in_maps = [
        {"inp": packed[i * BPC:(i + 1) * BPC]}
        for i in range(NCORES)
    ]
    res = run_bass_kernel_spmd(nc, in_maps, list(range(NCORES)))
    pbase = np.concatenate(
        [res.results[i]["pbase"] for i in range(NCORES)], axis=0
    ).astype(np.float32)
    u = np.concatenate(
        [res.results[i]["uv"] for i in range(NCORES)], axis=0
    ).astype(np.float32).transpose(0, 2, 1).reshape(B, S)
    z = np.concatenate(
        [res.results[i]["zv"] for i in range(NCORES)], axis=0
    ).astype(np.float32).reshape(B, S)
    # host Sherman-Morrison combine (f32)
    sdot = (z * mactf).sum(axis=1)
    delta = np.float32(1.0) - gamma * sdot
    kappa = (gamma / delta).astype(np.float32)
    zk = kappa[:, None] * z
    zk[:, 0] = 0.0
    Au = E * u[:, :, None]
    probs = pbase + Au * zk[:, :, None] - Au * zk[:, None, :]
    return probs.astype(np.float32)


# revision 51
# speedup vs baseline: 1.0189x; 1.0044x over previous
"""Matrix-Tree edge marginals on 8 Trainium2 NeuronCores.

probs[b,i,j] = d logZ / d scores[b,i,j] with logZ from the Matrix-Tree
theorem.  Closed form: with A = exp(masked scores - m) and Lfull the
(row/col-0-padded) Laplacian, probs = A ⊙ (diag(Y)·1^T − Y) where
Y = (Lfull^T)^{-1}.

Device算法 (per 256x256 matrix, 32 per core):
 1. Deflation: the Jacobi-preconditioned Laplacian has ONE slow outlier
    eigenvalue (Perron/root-escape mode) and a tight bulk (|1-λ| ≤ 0.09).
    The host adds γ·mact·mactᵀ (γ = mean_degree/n_active, bf16-exact)
    while building the Laplacian; the true inverse is recovered via a
    rank-1 Sherman-Morrison correction applied on the host.
 2. Host packs the deflated Laplacian Lt and A (it computes exp anyway),
    so device setup is just bf16 splits.
 3. Scaled-space Newton, round 1 in closed form: with G = rt∘Lh (bf16),
    B̄ = I-G, W1 = 2I-G, V1 = W1ᵀ (DMA-XBAR transpose):
    Q = B̄ᵀ@V1 = B+B², Yf1 = rt∘(I+Q) — one 256³ matmul.
 4. Round 2 polishes with the true split-bf16 residual (3-matmul
    Lh/Ll × Yh/Yl product) — needed for Sherman-Morrison denominator
    accuracy (the δ it feeds is a ~5e-3 cancellation).
 5. Device ships Pbase = A⊙(diag(Yf)1ᵀ − Yf) plus the row-sum vector
    u = Ỹ·mact (plain row sums — block-diagonal structure makes masking
    free) and column-sum vector z = mactᵀỸ (split-bf16 thin matmuls).
    Host finishes: δ = 1-γ·z·mact, κ = γ/δ, zk = κz (zk[0]=0: the root
    column of Ỹ is e0), P = Pbase + (A∘u)∘zk_i − (A∘u)∘zk_j.
"""

import numpy as np

import concourse.bass as bass
import concourse.bacc as bacc
import concourse.mybir as mybir
from concourse.bass import ds, ts
from concourse.masks import make_identity
from concourse.tile import TileContext
from concourse.bass_utils import run_bass_kernel_spmd

B, S, P = 256, 256, 128
NCORES = 8
BPC = B // NCORES   # matrices per core
RB = S // P         # row blocks per matrix
GRP = 6             # matrices interleaved per group
CGAMMA = 1.0        # deflation strength
NEG = np.float32(-1e9)

f32 = mybir.dt.float32
bf16 = mybir.dt.bfloat16
MULT = mybir.AluOpType.mult
ADD = mybir.AluOpType.add
SUB = mybir.AluOpType.subtract
AX = mybir.AxisListType.X
COPY = mybir.ActivationFunctionType.Copy
IDENT = mybir.ActivationFunctionType.Identity

OFF_LT = 0                  # RB*S: deflated Laplacian rows
OFF_A = RB * S              # RB*S: A = exp(s - m) rows
OFF_RT = 2 * RB * S         # 2: rt = 1/diag(Lt), column layout
PACK = OFF_RT + 2


def _mm256(nc, out_ps, lhsT, rhs):
    for I in range(RB):
        for K in range(RB):
            nc.tensor.matmul(
                out_ps[:, I, :],
                lhsT[:, K, ts(I, P)],
                rhs[:, K, :],
                start=(K == 0),
                stop=(K == RB - 1),
            )


def _mm256_acc(nc, out_ps, pairs):
    n = len(pairs) * RB
    for I in range(RB):
        cnt = 0
        for lhsT, rhs in pairs:
            for K in range(RB):
                nc.tensor.matmul(
                    out_ps[:, I, :],
                    lhsT[:, K, ts(I, P)],
                    rhs[:, K, :],
                    start=(cnt == 0),
                    stop=(cnt == n - 1),
                )
                cnt += 1


def build_program():
    nc = bacc.Bacc()
    inp = nc.dram_tensor("inp", [BPC, P, PACK], f32, kind="ExternalInput")
    out = nc.dram_tensor("pbase", [BPC, S, S], f32, kind="ExternalOutput")
    uv = nc.dram_tensor("uv", [BPC, P, RB], f32, kind="ExternalOutput")
    zv = nc.dram_tensor("zv", [BPC, 1, S], f32, kind="ExternalOutput")

    with TileContext(nc) as tc:
        with (
            tc.tile_pool(name="consts", bufs=1) as consts,
            tc.tile_pool(name="mat", bufs=3) as mat,
            tc.tile_pool(name="small", bufs=12) as small,
            tc.tile_pool(name="psT", bufs=2, space="PSUM") as ppT,
            tc.tile_pool(name="psD", bufs=2, space="PSUM") as ppD,
            tc.tile_pool(name="psbt", bufs=2, space="PSUM") as pbt,
            tc.tile_pool(name="psrow", bufs=2, space="PSUM") as prow,
        ):
            ident = consts.tile([P, P], f32)
            make_identity(nc, ident)
            identbig = consts.tile([P, RB, S], f32)
            nc.vector.memset(identbig, 0.0)
            for rb in range(RB):
                nc.vector.tensor_copy(identbig[:, rb, ts(rb, P)], ident)
            identbig_bf = consts.tile([P, RB, S], bf16)
            nc.scalar.activation(identbig_bf, identbig, COPY)
            i2bf = consts.tile([P, RB, S], bf16)
            nc.vector.tensor_scalar_mul(i2bf, identbig, 2.0)
            idbf = consts.tile([P, P], bf16)
            nc.scalar.activation(idbf, ident, COPY)
            onescol_bf = consts.tile([P, 1], bf16)
            nc.vector.memset(onescol_bf, 1.0)

            def setup(b):
                st = {}
                packed = mat.tile([P, PACK], f32, tag="packed", bufs=13)
                nc.sync.dma_start(packed, inp[b])
                st["packed"] = packed
                Ltp = packed[:, OFF_LT : OFF_LT + RB * S].rearrange(
                    "p (rb j) -> p rb j", rb=RB
                )
                st["Aa"] = packed[:, OFF_A : OFF_A + RB * S].rearrange(
                    "p (rb j) -> p rb j", rb=RB
                )
                rt = packed[:, OFF_RT : OFF_RT + 2]
                st["rt"] = rt

                Lh = mat.tile([P, RB, S], bf16, tag="Lh", bufs=13)
                nc.scalar.activation(Lh, Ltp, COPY)
                Ll = mat.tile([P, RB, S], bf16, tag="Ll", bufs=13)
                nc.gpsimd.tensor_sub(Ll, Ltp, Lh)
                st["Lh"], st["Ll"] = Lh, Ll

                G = mat.tile([P, RB, S], bf16, tag="G", bufs=7)
                for rb in range(RB):
                    nc.vector.tensor_scalar_mul(
                        G[:, rb, :], Lh[:, rb, :], rt[:, ds(rb, 1)]
                    )
                Bbar = mat.tile([P, RB, S], bf16, tag="Bbar", bufs=7)
                nc.vector.tensor_sub(Bbar, identbig_bf, G)
                W1 = mat.tile([P, RB, S], bf16, tag="W1", bufs=13)
                nc.gpsimd.tensor_sub(W1, i2bf, G)
                st["Bbar"], st["W1"] = Bbar, W1
                # V1 = W1^T via PE transpose (PSUM) + ACT copy to SBUF
                V1ps = pbt.tile([P, RB, S], bf16, tag="BT")
                for I in range(RB):
                    for K in range(RB):
                        nc.tensor.transpose(
                            V1ps[:, I, ts(K, P)], W1[:, K, ts(I, P)], idbf
                        )
                V1sb = mat.tile([P, RB, S], bf16, tag="V1", bufs=7)
                nc.scalar.activation(V1sb, V1ps, COPY)
                st["V1"] = V1sb
                return st

            def round1(st):
                rt = st["rt"]
                Qps = ppD.tile([P, RB, S], f32, tag="dY")
                _mm256(nc, Qps, st["Bbar"], st["V1"])
                S1 = mat.tile([P, RB, S], f32, tag="Yf", bufs=13)
                nc.vector.tensor_add(S1, identbig, Qps)
                for rb in range(RB):
                    nc.scalar.mul(S1[:, rb, :], S1[:, rb, :], rt[:, ds(rb, 1)])
                st["Yf"] = S1

            def round2a(st):
                Yh2 = mat.tile([P, RB, S], bf16, tag="Yh", bufs=8)
                nc.scalar.activation(Yh2, st["Yf"], COPY)
                Yl2 = mat.tile([P, RB, S], bf16, tag="Yl", bufs=8)
                nc.gpsimd.tensor_sub(Yl2, st["Yf"], Yh2)
                Tps = ppT.tile([P, RB, S], f32, tag="T")
                _mm256_acc(
                    nc, Tps,
                    [(st["Lh"], Yh2), (st["Lh"], Yl2), (st["Ll"], Yh2)],
                )
                st["Tps"] = Tps

            def round2b(st):
                rt = st["rt"]
                R2 = mat.tile([P, RB, S], bf16, tag="R", bufs=4)
                nc.vector.tensor_sub(R2, identbig, st["Tps"])
                dY2ps = ppD.tile([P, RB, S], f32, tag="dY")
                _mm256(nc, dY2ps, st["W1"], R2)
                tupd = mat.tile([P, RB, S], f32, tag="tupd", bufs=4)
                for rb in range(RB):
                    nc.scalar.mul(tupd[:, rb, :], dY2ps[:, rb, :], rt[:, ds(rb, 1)])
                nc.gpsimd.tensor_add(st["Yf"], st["Yf"], tupd)

            def sm_out(b, st):
                Yf, Aa = st["Yf"], st["Aa"]
                # z = onesᵀYf via split-bf16 thin matmuls (PSUM accumulates)
                Yh3 = mat.tile([P, RB, S], bf16, tag="Yh", bufs=8)
                nc.scalar.activation(Yh3, Yf, COPY)
                Yl3 = mat.tile([P, RB, S], bf16, tag="Yl", bufs=8)
                nc.vector.tensor_sub(Yl3, Yf, Yh3)
                zps = prow.tile([1, S], f32, tag="srow")
                cnt = 0
                for piece in (Yh3, Yl3):
                    for rb in range(RB):
                        nc.tensor.matmul(
                            zps, onescol_bf, piece[:, rb, :],
                            start=(cnt == 0), stop=(cnt == 2 * RB - 1),
                        )
                        cnt += 1
                # u = row sums; z row copied to SBUF for DMA
                uz = small.tile([P, RB], f32, tag="uz", bufs=5)
                nc.vector.tensor_reduce(uz, Yf, AX, ADD)
                zsb = small.tile([1, S], f32, tag="zsb", bufs=5)
                nc.scalar.activation(zsb, zps, COPY)
                # dg = diag(Yf); Pbase = A ⊙ (dg_i - Yf)
                dg = small.tile([P, RB], f32, tag="dg")
                for rb in range(RB):
                    scr = small.tile([P, P], f32, tag="scr", bufs=4)
                    nc.gpsimd.tensor_mul(scr, ident, Yf[:, rb, ts(rb, P)])
                    nc.vector.tensor_reduce(dg[:, ds(rb, 1)], scr, AX, ADD)
                t3 = mat.tile([P, RB, S], f32, tag="t3", bufs=4)
                for rb in range(RB):
                    nc.scalar.activation(
                        t3[:, rb, :], Yf[:, rb, :], IDENT,
                        bias=dg[:, ds(rb, 1)], scale=-1.0,
                    )
                Pr = mat.tile([P, RB, S], f32, tag="Pr", bufs=5)
                if b % 2 == 0:
                    nc.vector.tensor_mul(Pr, t3, Aa)
                else:
                    nc.gpsimd.tensor_mul(Pr, t3, Aa)
                nc.sync.dma_start(
                    out[b].rearrange("(rb p) j -> p rb j", p=P), Pr
                )
                nc.sync.dma_start(uv[b], uz)
                nc.sync.dma_start(zv[b], zsb[0:1, :])

            groups = [
                list(range(g0, min(g0 + GRP, BPC)))
                for g0 in range(0, BPC, GRP)
            ]
            sts = {}
            for b in groups[0]:
                sts[b] = setup(b)
            for gi, grp in enumerate(groups):
                nxt = groups[gi + 1] if gi + 1 < len(groups) else []
                for b in grp:
                    round1(sts[b])
                for b in grp:
                    round2a(sts[b])
                for b in grp:
                    round2b(sts[b])
                for b in nxt:
                    sts[b] = setup(b)
                for b in grp:
                    sm_out(b, sts[b])
                    del sts[b]
    nc.finalize()
    return nc


_prog = None


def _get_program():
    global _prog
    if _prog is None:
        _prog = build_program()
    return _prog


def _bf16_exact(x):
    u = np.asarray(x, dtype=np.float32).view(np.uint32)
    u = (u + 0x8000) & 0xFFFF0000
    return u.view(np.float32)


def _host_prep(scores, mask):
    scores = np.asarray(scores, dtype=np.float32)
    mask = np.asarray(mask).astype(bool)
    mr = mask.copy()
    mr[:, 0] = True
    pair = mr[:, :, None] & mr[:, None, :]
    spre = np.where(pair, scores, NEG)
    spre[:, 0, :] = NEG
    m = spre.max(axis=(1, 2))                      # [B]
    E = np.exp(np.clip(spre - m[:, None, None], -80.0, 0.0), dtype=np.float32)
    E[:, 0, :] = 0.0
    d = E.sum(axis=2)                              # [B, S]
    mactf = mask.astype(np.float32)
    n_act = mactf.sum(axis=1)
    dbar = (d * mactf).sum(axis=1) / n_act
    gamma = _bf16_exact(CGAMMA * dbar / n_act)     # [B], bf16-exact

    Lt = -E.copy()
    idx = np.arange(S)
    Lt[:, idx, idx] += d
    Lt += gamma[:, None, None] * (mactf[:, :, None] * mactf[:, None, :])
    Lt = np.where(mr[:, :, None], Lt, np.eye(S, dtype=np.float32)[None])
    Lt[:, :, 0] = 0.0
    Lt[:, 0, :] = 0.0
    Lt[:, 0, 0] = 1.0
    Lt = Lt.astype(np.float32)
    diagL = np.einsum('bii->bi', Lt)
    rt = (np.float32(1.0) / diagL).astype(np.float32)

    def colmaj(v):
        return v.reshape(B, RB, P).transpose(0, 2, 1)

    def rowpack(M):
        return M.reshape(B, RB, P, S).transpose(0, 2, 1, 3).reshape(B, P, RB * S)

    packed = np.zeros((B, P, PACK), dtype=np.float32)
    packed[:, :, OFF_LT : OFF_LT + RB * S] = rowpack(Lt)
    packed[:, :, OFF_A : OFF_A + RB * S] = rowpack(E)
    packed[:, :, OFF_RT : OFF_RT + 2] = colmaj(rt)
    return packed, E, mactf, gamma


def kernel(scores, mask):
    packed, E, mactf, gamma = _host_prep(scores, mask)
    nc = _get_program()
    in_maps = [
        {"inp": packed[i * BPC:(i + 1) * BPC]}
        for i in range(NCORES)
    ]
    res = run_bass_kernel_spmd(nc, in_maps, list(range(NCORES)))
    pbase = np.concatenate(
        [res.results[i]["pbase"] for i in range(NCORES)], axis=0
    ).astype(np.float32)
    u = np.concatenate(
        [res.results[i]["uv"] for i in range(NCORES)], axis=0
    ).astype(np.float32).transpose(0, 2, 1).reshape(B, S)
    z = np.concatenate(
        [res.results[i]["zv"] for i in range(NCORES)], axis=0
    ).astype(np.float32).reshape(B, S)
    # host Sherman-Morrison combine (f32)
    sdot = (z * mactf).sum(axis=1)
    delta = np.float32(1.0) - gamma * sdot
    kappa = (gamma / delta).astype(np.float32)
    zk = kappa[:, None] * z
    zk[:, 0] = 0.0
    Au = E * u[:, :, None]
    probs = pbase + Au * zk[:, :, None] - Au * zk[:, None, :]
    return probs.astype(np.float32)
